# revision 16
# baseline (speedup 1.0000x reference)
"""Causal single-head attention (B=4, T=4096, C=1024, H=64) on 8 trn2 cores.

Sharding: 2 cores per batch element, 2048 queries each. Window trick for
SPMD uniformity: every core sees a 4096-wide key window with its queries at
window positions [2048, 4096). Core h of batch b gets keys x[b, 0:2048*(h+1)]
right-aligned (h=0: first 2048 key columns zero, killed via a zeroed
ones-column so they contribute nothing to numerator or denominator). Both
cores run one identical program with exact causal masking in window coords.

Perf structure:
  - matmuls in fp32r (single-pass full-rate fp32 on the PE)
  - projections col-packed: two 512-column t-blocks computed concurrently on
    array column halves -> psum partitions [0:64] / [64:128]
  - K^T/Q^T stored double-stacked on partitions; scores row-packed: two
    key tiles (j, j+4) computed concurrently on array row halves into one
    [128, 1024] psum pair, one exp (ACT) per pair
  - attention software-pipelined one step so the PE never waits on ACT
  - causal diagonal masked post-exp with GPSIMD affine_select
  - O'^T accumulated with a fused ones-column denominator, finalized by PE
    transpose + DVE reciprocal/scale
"""

import numpy as np
import ml_dtypes

import concourse.bass as bass
import concourse.bacc as bacc
import concourse.tile as tile
from concourse import mybir
from concourse.bass_utils import run_bass_kernel_spmd

B, T, C, H = 4, 4096, 1024, 64
N_CORES = 8
TQ = 2048            # queries per core
NQB = 4              # q-blocks of 512
NCH = C // 128       # 8 contraction chunks
NTB = T // 512       # 8 key t-blocks
NKT = T // 128       # 32 key tiles
F32 = mybir.dt.float32
F32R = mybir.dt.float32r
BF16 = mybir.dt.bfloat16

_nc_cache = {}


def build_module():
    if "nc" in _nc_cache:
        return _nc_cache["nc"]
    nc = bacc.Bacc("TRN2", target_bir_lowering=False, debug=False,
                   num_devices=N_CORES)
    xk = nc.dram_tensor("xk", [C, T], BF16, kind="ExternalInput").ap()
    wq = nc.dram_tensor("wq", [C, H], BF16, kind="ExternalInput").ap()
    wk = nc.dram_tensor("wk", [C, H], BF16, kind="ExternalInput").ap()
    wv = nc.dram_tensor("wv", [C, H], BF16, kind="ExternalInput").ap()
    ones_kv = nc.dram_tensor("ones_kv", [128, NKT], F32,
                             kind="ExternalInput").ap()
    ident = nc.dram_tensor("ident", [128, 128], F32, kind="ExternalInput").ap()
    ident2 = nc.dram_tensor("ident2", [128, 64], BF16,
                            kind="ExternalInput").ap()
    out = nc.dram_tensor("out", [TQ, H], F32, kind="ExternalOutput").ap()

    with tile.TileContext(nc) as tc:
        with (
            tc.tile_pool(name="consts", bufs=1) as consts,
            tc.tile_pool(name="xt", bufs=2) as xt_pool,
            tc.tile_pool(name="vtmp", bufs=2) as vtmp_pool,
            tc.tile_pool(name="exps", bufs=4) as exps_pool,
            tc.tile_pool(name="fin", bufs=2) as fin_pool,
            tc.tile_pool(name="ps_s", bufs=2, space="PSUM") as ps_s,
            tc.tile_pool(name="ps_o", bufs=2, space="PSUM") as ps_o,
            tc.tile_pool(name="ps_p", bufs=2, space="PSUM") as ps_p,
        ):
            # ---- constants / weights in SBUF ----
            w_sb = {}
            for name, ap in (("wq", wq), ("wk", wk), ("wv", wv)):
                t = consts.tile([128, NCH, H], BF16, name=f"{name}_sb")
                nc.sync.dma_start(t[:], ap.rearrange("(ch p) h -> p ch h", p=128))
                w_sb[name] = t
            id_sb = consts.tile([128, 128], F32, name="id_sb")
            nc.sync.dma_start(id_sb[:], ident)
            id2_sb = consts.tile([128, 64], BF16, name="id2_sb")
            nc.sync.dma_start(id2_sb[:], ident2)
            ones_sb = consts.tile([128, NKT], F32, name="ones_sb")
            nc.sync.dma_start(ones_sb[:], ones_kv)

            # ---- persistent activations ----
            # kt2x: pair-group pg holds K^T for t-blocks (2pg, 2pg+1) on
            # partition halves [0:64] / [64:128], columns pg*512 + w.
            kt2x = consts.tile([128, TQ], BF16, name="kt2x")
            # qt2x: Q^T duplicated on both partition halves.
            qt2x = consts.tile([128, TQ], BF16, name="qt2x")
            v_all = consts.tile([128, NKT, H + 1], BF16, name="v_all")

            nc.vector.tensor_copy(v_all[:, :, H], ones_sb[:])

            xk_r = xk.rearrange("(ch p) t -> p ch t", p=128)

            inv_sqrt_h = 1.0 / np.sqrt(np.float32(H))

            # ---- issue all xk loads up front; query-bearing pair first ----
            xt_tiles = {}
            for tb0 in (4, 0, 2, 6):
                xt = xt_pool.tile([128, NCH, 1024], BF16, tag=f"xt{tb0}",
                                  name=f"xt{tb0}")
                nc.sync.dma_start(xt[:, :, 0:512],
                                  xk_r[:, :, tb0 * 512:(tb0 + 1) * 512])
                nc.sync.dma_start(xt[:, :, 512:1024],
                                  xk_r[:, :, (tb0 + 1) * 512:(tb0 + 2) * 512])
                xt_tiles[tb0] = xt

            def kt_slice(j):
                tb, s = j // 4, j % 4
                half, pg = tb % 2, tb // 2
                return kt2x[64 * half:64 * (half + 1),
                            pg * 512 + s * 128: pg * 512 + (s + 1) * 128]

            def proj_pair(tb0, wname, pdst):
                xa = xt_tiles[tb0][:, :, 0:512]
                xb = xt_tiles[tb0][:, :, 512:1024]
                for ch in range(NCH):
                    nc.tensor.matmul(pdst[0:64, :], w_sb[wname][:, ch, :],
                                     xa[:, ch, :],
                                     start=(ch == 0), stop=(ch == NCH - 1))
                for ch in range(NCH):
                    nc.tensor.matmul(pdst[64:128, :], w_sb[wname][:, ch, :],
                                     xb[:, ch, :],
                                     start=(ch == 0), stop=(ch == NCH - 1),
                                     tile_position=(0, 64))
                return pdst

            def emit_projQ(tb0):
                qoff = 0 if tb0 == 4 else 1024
                pq = proj_pair(tb0, "wq",
                               ps_p.tile([128, 512], F32, tag="pp",
                                         name=f"pq{tb0}"))
                for half in range(2):
                    sl = pq[64 * half:64 * (half + 1), :]
                    dst = slice(qoff + half * 512, qoff + (half + 1) * 512)
                    nc.vector.tensor_copy(qt2x[0:64, dst], sl)
                    nc.vector.tensor_copy(qt2x[64:128, dst], sl)

            def emit_projK(tb0):
                pg = tb0 // 2
                pk = proj_pair(tb0, "wk",
                               ps_p.tile([128, 512], F32, tag="pp",
                                         name=f"pk{tb0}"))
                nc.vector.tensor_copy(kt2x[:, pg * 512:(pg + 1) * 512], pk[:])

            def emit_projV(tb0):
                pv = proj_pair(tb0, "wv",
                               ps_p.tile([128, 512], F32, tag="pp",
                                         name=f"pv{tb0}"))
                vt = vtmp_pool.tile([128, 512], BF16, tag="vt",
                                    name=f"vt{tb0}")
                nc.vector.tensor_copy(vt[:], pv[:])
                for half in range(2):
                    for s in range(4):
                        j = 4 * (tb0 + half) + s
                        ptr = ps_p.tile([128, 64], BF16, tag="pp",
                                        name=f"ptr{j}")
                        nc.tensor.transpose(
                            ptr[:],
                            vt[64 * half:64 * (half + 1),
                               s * 128:(s + 1) * 128],
                            id2_sb[64 * half:64 * (half + 1), :])
                        nc.vector.tensor_copy(v_all[:, j, 0:H], ptr[:])

            attn_state = {}

            def attn_begin(qb):
                po = ps_o.tile([H + 1, 512], F32, tag="po", name=f"po{qb}")
                attn_state[qb] = dict(po=po, queue=[], jmax=20 + 4 * qb)

            def attn_flush_one(qb):
                st = attn_state[qb]
                js, es2 = st["queue"].pop(0)
                jmax = st["jmax"]
                for idx, j in enumerate(js):
                    nc.tensor.matmul(
                        st["po"][:], v_all[:, j, :],
                        es2[:, idx * 512:(idx + 1) * 512],
                        start=(j == 0), stop=(j == jmax - 1),
                        skip_group_check=True)

            def attn_items(qb, items):
                st = attn_state[qb]
                jmax = st["jmax"]
                diag0 = jmax - 4
                qs_a = qt2x[0:64, qb * 512:(qb + 1) * 512]
                qs_b = qt2x[64:128, qb * 512:(qb + 1) * 512]
                for jA, jB in items:
                    ps = ps_s.tile([128, 1024], F32, tag="ps",
                                   name=f"s{qb}_{jA}")
                    nc.tensor.matmul(ps[:, 0:512], kt_slice(jA),
                                     qs_a, start=True, stop=True)
                    es2 = exps_pool.tile([128, 1024], BF16, tag="es",
                                         name=f"e{qb}_{jA}")
                    if jB is not None:
                        nc.tensor.matmul(ps[:, 512:1024], kt_slice(jB),
                                         qs_b, start=True, stop=True,
                                         tile_position=(64, 0))
                        nc.scalar.activation(es2[:], ps[:],
                                             mybir.ActivationFunctionType.Exp,
                                             scale=float(inv_sqrt_h))
                        if jB >= diag0:
                            d = jB - diag0
                            nc.gpsimd.affine_select(
                                es2[:, 512:1024], es2[:, 512:1024],
                                pattern=[[1, 512]],
                                compare_op=mybir.AluOpType.is_ge,
                                fill=0.0, base=-(128 * d),
                                channel_multiplier=-1)
                        st["queue"].append(((jA, jB), es2))
                    else:
                        nc.scalar.activation(es2[:, 0:512], ps[:, 0:512],
                                             mybir.ActivationFunctionType.Exp,
                                             scale=float(inv_sqrt_h))
                        if jA >= diag0:
                            d = jA - diag0
                            nc.gpsimd.affine_select(
                                es2[:, 0:512], es2[:, 0:512],
                                pattern=[[1, 512]],
                                compare_op=mybir.AluOpType.is_ge,
                                fill=0.0, base=-(128 * d),
                                channel_multiplier=-1)
                        st["queue"].append(((jA,), es2))
                    if len(st["queue"]) > 1:
                        attn_flush_one(qb)

            def attn_flush(qb):
                st = attn_state[qb]
                while st["queue"]:
                    attn_flush_one(qb)

            def attn_final(qb):
                st = attn_state[qb]
                ot = fin_pool.tile([H + 1, 512], F32, tag="ot", name=f"ot{qb}")
                nc.vector.tensor_copy(ot[:], st["po"][:])
                ob = fin_pool.tile([128, 4, H], F32, tag="ob", name=f"ob{qb}")
                for s in range(4):
                    ptr = ps_p.tile([128, H + 1], F32, tag="pp",
                                    name=f"otr{qb}_{s}")
                    nc.tensor.transpose(ptr[:], ot[:, s * 128:(s + 1) * 128],
                                        id_sb[:H + 1, :H + 1])
                    rc = fin_pool.tile([128, 1], F32, tag="rc",
                                       name=f"rc{qb}_{s}")
                    nc.vector.reciprocal(rc[:], ptr[:, H:H + 1])
                    nc.vector.tensor_scalar_mul(ob[:, s, :], ptr[:, 0:H], rc[:])
                nc.sync.dma_start(
                    out[qb * 512:(qb + 1) * 512, :].rearrange(
                        "(s p) h -> p s h", p=128),
                    ob[:])

            def G(g):
                return [(8 * g + s, 8 * g + 4 + s) for s in range(4)]

            # ---- interleaved emission schedule ----
            # Constraint (program order = dataflow): projK/projQ before the
            # scores that read them; projV before the (lag-1 deferred)
            # O-matmuls that read its v_all tiles.
            emit_projQ(4)
            emit_projK(4)
            emit_projK(0)
            emit_projV(0)
            emit_projV(4)
            attn_begin(0)
            attn_items(0, G(0))
            emit_projK(2)
            emit_projV(2)
            attn_items(0, G(1))
            attn_items(0, [(j, None) for j in range(16, 20)])
            attn_flush(0)
            attn_begin(1)
            attn_items(1, G(0))
            attn_final(0)
            attn_items(1, G(1))
            emit_projQ(6)
            attn_items(1, G(2))
            attn_flush(1)
            emit_projK(6)
            attn_begin(2)
            attn_items(2, G(0))
            attn_final(1)
            attn_items(2, G(1))
            emit_projV(6)
            attn_items(2, G(2))
            attn_items(2, [(j, None) for j in range(24, 28)])
            attn_flush(2)
            attn_begin(3)
            attn_items(3, G(0))
            attn_final(2)
            for g in range(1, 4):
                attn_items(3, G(g))
            attn_flush(3)
            attn_final(3)
    nc.compile()
    _nc_cache["nc"] = nc
    return nc


def _core_inputs(x, Wq, Wk, Wv, core):
    b, h = core // 2, core % 2
    xkm = np.zeros((C, T), dtype=np.float32)
    nk = 2048 * (h + 1)
    xkm[:, T - nk:] = x[b, 0:nk, :].T
    ones = np.zeros((128, NKT), dtype=np.float32)
    ones[:, (T - nk) // 128:] = 1.0
    id2 = np.zeros((128, 64), dtype=np.float32)
    id2[:64] = np.eye(64, dtype=np.float32)
    id2[64:] = np.eye(64, dtype=np.float32)
    bf = ml_dtypes.bfloat16
    return {
        "xk": np.ascontiguousarray(xkm.astype(bf)),
        "wq": np.ascontiguousarray(np.asarray(Wq, dtype=np.float32).astype(bf)),
        "wk": np.ascontiguousarray(np.asarray(Wk, dtype=np.float32).astype(bf)),
        "wv": np.ascontiguousarray(np.asarray(Wv, dtype=np.float32).astype(bf)),
        "ones_kv": ones,
        "ident": np.eye(128, dtype=np.float32),
        "ident2": id2.astype(bf),
    }


def kernel(x, Wq, Wk, Wv):
    x = np.asarray(x, dtype=np.float32)
    nc = build_module()
    in_maps = [_core_inputs(x, Wq, Wk, Wv, c) for c in range(N_CORES)]
    res = run_bass_kernel_spmd(nc, in_maps, core_ids=list(range(N_CORES)))
    out = np.empty((B, T, H), dtype=np.float32)
    for core in range(N_CORES):
        b, h = core // 2, core % 2
        out[b, 2048 * h:2048 * (h + 1), :] = res.results[core]["out"]
    return out
